# revision 30
# baseline (speedup 1.0000x reference)
"""Trainium2 Bass kernel for nn_Attention_81870666597078.

Multi-head causal self-attention (b=4, s=2048, d=1024, 16 heads) with QKV/O
projections. Hybrid sharding over 8 NeuronCores: core c handles batch c//2
and head-group c%2 (8 heads = 512 hidden dims). Each core produces a partial
O projection [2048, 1024] for its batch; the host sums the 2 partials per
batch ("all-reduce").

Per-core dataflow (matmuls in fp16 with fp32 PSUM accumulation):
  - QKV projection into transposed layout qT/kT/vT [128, hp, seq] per
    head-pair hp (4 pairs of heads; a pair = 128 dims).
  - vT re-transposed on the PE into v-natural [seq, dh] tiles with a fused
    ones-column ([v | 1] per head per 128-k block) so the PV matmul also
    produces the softmax denominator as output row 64.
  - Scores computed transposed, S^T [k, q]; both heads of a pair share one
    [128, 1024] PSUM tile (K=64 at base partitions 0/64 -> concurrent PE row
    groups). One ACT exp covers both heads; on diagonal tiles the exp is
    sliced to skip the fully-masked column range. No max subtraction (scores
    ~N(0,1) after the 1/8 scale). Causal masking multiplies a 0/1 [128,128]
    mask on the diagonal band; fully-masked ranges are skipped via slicing.
  - The kt loop is software-pipelined (scores for kt+1 issue before PV of kt)
    and independent PE work units (QKV projection of the next head-pair,
    v-transposes, O projection of completed q tiles) are interleaved into the
    exp-latency gaps to keep the PE dense and the HAM clock warm.
  - Normalization: the denominator row is copied out of PSUM, reciprocated on
    the DVE, and broadcast across 64 partitions on the otherwise-idle GpSimd
    engine (partition_broadcast), keeping the PE/ACT/DVE out of that chain.
  - O projection accumulates the 4 head-pair contributions in PSUM (K=512
    total) and writes fp16 partials; the host sums in fp32.
"""
import os
from collections import deque

import numpy as np

import concourse.bass as bass  # noqa: F401
import concourse.mybir as mybir
from concourse import bacc
from concourse.bass_utils import run_bass_kernel_spmd
from concourse.masks import make_identity
from concourse.tile import TileContext

dt = mybir.dt
F32 = dt.float32
F16 = dt.float16
Exp = mybir.ActivationFunctionType.Exp

N_CORES = 8
B = 4
S = 2048          # seq per core (one batch)
D = 1024
DH = 64
NHP = 4           # head-pairs per core (8 heads, 512 dims)
CD = 512          # head dims per core
NDT = D // 128    # 8 k-tiles over the model dim
NST = S // 512    # 4 seq tiles of 512
NTT = S // 128    # 16 seq tiles of 128


def _build_bass():
    nc = bacc.Bacc("TRN2", target_bir_lowering=False, debug=False)
    # inputs pre-swizzled on the host into the exact SBUF layouts so every
    # DMA is one large contiguous descriptor per partition
    xt = nc.dram_tensor("xt", [128, NST, NDT, 512], F16, kind="ExternalInput")
    wqkvt = nc.dram_tensor("wqkvt", [128, NHP, NDT, 384], F16,
                           kind="ExternalInput")
    wot = nc.dram_tensor("wot", [128, NHP, D], F16, kind="ExternalInput")
    mask = nc.dram_tensor("mask", [128, 128], F16, kind="ExternalInput")
    onesd = nc.dram_tensor("onesd", [128, 64], F32, kind="ExternalInput")
    out = nc.dram_tensor("out", [S, D], F16, kind="ExternalOutput")

    with TileContext(nc) as tc:
        with (
            tc.tile_pool(name="const", bufs=1) as const,
            tc.tile_pool(name="big", bufs=1) as big,
            tc.tile_pool(name="probs", bufs=8) as probsp,
            tc.tile_pool(name="outp", bufs=4) as outp,
            tc.tile_pool(name="small", bufs=3) as small,
            tc.tile_pool(name="psS", bufs=2, space="PSUM") as psS,
            tc.tile_pool(name="psPV", bufs=2, space="PSUM") as psPV,
            tc.tile_pool(name="psA", bufs=2, space="PSUM") as psA,
        ):
            wq_sb = const.tile([128, NHP, NDT, 384], F16, tag="wq")
            wot_sb = const.tile([128, NHP, D], F16, tag="wot")
            mask_sb = const.tile([128, 128], F16, tag="mask")
            ident_sb = const.tile([128, 128], F16, tag="ident")
            ones_sb = const.tile([128, 64], F32, tag="ones")
            xts = big.tile([128, NST, NDT, 512], F16, tag="xts")
            nc.sync.dma_start(xts[:, 0], xt.ap()[:, 0])
            # head-pair 0's weights first so QKV can start ASAP
            for hp in range(NHP):
                nc.sync.dma_start(wq_sb[:, hp], wqkvt.ap()[:, hp])
            for st in range(1, NST):
                nc.sync.dma_start(xts[:, st], xt.ap()[:, st])
            nc.sync.dma_start(wot_sb[:], wot.ap())
            nc.sync.dma_start(mask_sb[:], mask.ap())
            nc.sync.dma_start(ones_sb[:], onesd.ap())
            make_identity(nc, ident_sb[:])

            qT = big.tile([128, NHP, S], F16, tag="qT")
            kT = big.tile([128, NHP, S], F16, tag="kT")
            vT = big.tile([128, NHP, S], F16, tag="vT")
            aoT = big.tile([128, NHP, S], F16, tag="aoT")
            v65 = big.tile([128, NHP * NTT * 2 * 65], F16, tag="v65")
            v65v = v65[:].rearrange("p (a t h c) -> p a t h c", a=NHP, t=NTT, h=2)
            # ones column of every [v | 1] group
            nc.vector.tensor_copy(
                v65v[:, :, :, :, 64:65],
                ones_sb[:, 0:1][:, None, None, None, :]
                .broadcast_to([128, NHP, NTT, 2, 1]))

            dests = (qT, kT, vT)

            def qkv_unit(hp, st, g):
                ps = psA.tile([128, 512], F32, tag="psA")
                w0 = g * 128
                for kt in range(NDT):
                    nc.tensor.matmul(
                        ps[:], wq_sb[:, hp, kt, w0:w0 + 128], xts[:, st, kt, :],
                        start=(kt == 0), stop=(kt == NDT - 1))
                nc.vector.tensor_copy(
                    dests[g][:, hp, st * 512:(st + 1) * 512], ps[:])

            def transp_unit(hp, t):
                pst = psA.tile([128, 128], F16, tag="psA")
                nc.tensor.transpose(pst[:], vT[:, hp, t * 128:(t + 1) * 128],
                                    ident_sb[:])
                nc.vector.tensor_copy(
                    v65v[:, hp, t, :, 0:64],
                    pst[:].rearrange("p (h c) -> p h c", h=2))

            def oproj_unit(tt, ot):
                po = psA.tile([128, 512], F32, tag="psA")
                for c in range(NHP):
                    nc.tensor.matmul(
                        po[:], aoT[:, c, tt * 128:(tt + 1) * 128],
                        wot_sb[:, c, ot * 512:(ot + 1) * 512],
                        start=(c == 0), stop=(c == NHP - 1))
                ob = outp.tile([128, 512], F16, tag="ob")
                if (tt + ot) % 2 == 0:
                    nc.vector.tensor_copy(ob[:], po[:])
                else:
                    nc.scalar.copy(ob[:], po[:])
                nc.sync.dma_start(
                    out.ap()[tt * 128:(tt + 1) * 128, ot * 512:(ot + 1) * 512],
                    ob[:])

            def gen_units(hp):
                for st in range(NST):
                    for g in range(3):
                        yield (qkv_unit, hp, st, g)
                for t in range(NTT):
                    yield (transp_unit, hp, t)

            fill = deque()
            fill_budget = [0.0]

            def drain_fill(steps_left):
                # Spread the remaining units over the remaining kt steps of
                # this head-pair (whole units only; the remainder carries).
                if steps_left <= 0:
                    n = len(fill)
                else:
                    fill_budget[0] += len(fill) / steps_left
                    if fill_budget[0] < 1.0:
                        return
                    n = min(3, int(fill_budget[0]))
                    fill_budget[0] -= n
                for _ in range(min(n, len(fill))):
                    f = fill.popleft()
                    f[0](*f[1:])

            # head-pair 0's inputs are needed immediately
            for u in gen_units(0):
                u[0](*u[1:])

            def score_pair(hp, qt, kt):
                o = kt * 128 - qt * 512
                c0 = max(0, o)
                sp = psS.tile([128, 1024], F32, tag="s")
                pr = probsp.tile([128, 1024], F16, tag="pr")
                for h in (0, 1):
                    nc.tensor.matmul(
                        sp[:, h * 512 + c0:(h + 1) * 512],
                        kT[h * 64:(h + 1) * 64, hp, kt * 128:(kt + 1) * 128],
                        qT[h * 64:(h + 1) * 64, hp,
                           qt * 512 + c0:(qt + 1) * 512],
                        start=True, stop=True)
                return (sp, pr, o, c0)

            for hp in range(NHP):
                if hp + 1 < NHP:
                    fill.extend(gen_units(hp + 1))
                steps_left = sum(4 * (q + 1) for q in range(NST))
                for qt in range(NST):
                    # stage PE work into the qt-boundary window (the next PV
                    # group waits on the first exp; without queued fill the
                    # PE idles here and the HAM clock re-throttles)
                    for _ in range(2):
                        if fill:
                            f = fill.popleft()
                            f[0](*f[1:])
                    pv0 = psPV.tile([65, 512], F32, tag="pv")
                    pv1 = psPV.tile([65, 512], F32, tag="pv")
                    pvs = (pv0, pv1)
                    nkt = 4 * (qt + 1)
                    cur = score_pair(hp, qt, 0)
                    for kt in range(nkt):
                        nxt = (score_pair(hp, qt, kt + 1)
                               if kt + 1 < nkt else None)
                        sp, pr, o, c0 = cur
                        if o > 0:
                            for h in (0, 1):
                                nc.scalar.activation(
                                    pr[:, h * 512 + c0:(h + 1) * 512],
                                    sp[:, h * 512 + c0:(h + 1) * 512],
                                    Exp, scale=0.125)
                        else:
                            nc.scalar.activation(pr[:], sp[:], Exp, scale=0.125)
                        if o >= 0:
                            for h in (0, 1):
                                nc.vector.tensor_mul(
                                    pr[:, h * 512 + o:h * 512 + o + 128],
                                    pr[:, h * 512 + o:h * 512 + o + 128],
                                    mask_sb[:])
                        drain_fill(steps_left)
                        steps_left -= 1
                        for h in (0, 1):
                            nc.tensor.matmul(
                                pvs[h][:, c0:512],
                                v65v[:, hp, kt, h, :],
                                pr[:, h * 512 + c0:(h + 1) * 512],
                                start=(kt == 0), stop=(kt == nkt - 1),
                                skip_group_check=True)
                        cur = nxt
                    # normalize by the denominator (PV row 64)
                    for h in (0, 1):
                        pv = pvs[h]
                        den = small.tile([1, 512], F32, tag="den")
                        nc.vector.tensor_copy(den[:], pv[64:65, :])
                        rcf = small.tile([1, 512], F32, tag="rcf")
                        nc.vector.reciprocal_approx_fast(rcf[:], den[:])
                        # broadcast the reciprocal row across 64 partitions on
                        # the (idle) GpSimd engine instead of a K=1 PE matmul
                        rb = small.tile([64, 512], F32, tag="rb")
                        nc.gpsimd.partition_broadcast(rb[:], rcf[0:1, :],
                                                      channels=64)
                        nc.vector.tensor_mul(
                            aoT[h * 64:(h + 1) * 64, hp,
                                qt * 512:(qt + 1) * 512],
                            pv[0:64, :], rb[:])
                    if hp == NHP - 1:
                        # O projection for this q block now has all head pairs
                        for j in range(4):
                            for ot in range(2):
                                fill.append((oproj_unit, qt * 4 + j, ot))
            drain_fill(0)
    nc.compile()
    return nc


def _causal_mask():
    # mask[r, j] = 1 where key row r is visible to query column j
    r = np.arange(128)[:, None]
    j = np.arange(128)[None, :]
    return (r <= j).astype(np.float32)


def _maybe_register_ntff_hook():
    try:
        import antenv
        if getattr(antenv, "axon_hooks", None) is not None:
            return True
        import sys
        import types
        from trn_agent_boot.trn_boot import _ntff_profile_via_ctypes
        mod = types.ModuleType("antenv.axon_hooks")
        state = {"hook": _ntff_profile_via_ctypes("/opt/axon/libaxon_pjrt.so")}
        mod.set_axon_ntff_profile_hook = lambda h: state.__setitem__("hook", h)
        mod.get_axon_ntff_profile_hook = lambda: state["hook"]
        sys.modules["antenv.axon_hooks"] = mod
        antenv.axon_hooks = mod
        return True
    except Exception:
        return False


_NC_CACHE = {}


def kernel(x, W_qkv, W_o):
    assert x.shape == (B, S, D)
    mask = _causal_mask().astype(np.float16)
    onesd = np.ones((128, 64), dtype=np.float32)
    Wq, Wk, Wv = (np.asarray(W_qkv[i * D:(i + 1) * D]) for i in range(3))
    in_maps = []
    for c in range(N_CORES):
        b, hg = c // 2, c % 2
        # x[b]^T [D, S] -> [p, st, a, 512] so each partition's DMA payload is
        # one contiguous 8KB block per seq tile
        xt = np.asarray(x[b]).T.astype(np.float16)
        xt = np.ascontiguousarray(
            xt.reshape(NDT, 128, NST, 512).transpose(1, 2, 0, 3))
        rows = []
        for hp in range(NHP):
            d0 = hg * 512 + hp * 128
            for W in (Wq, Wk, Wv):
                rows.append(W[d0:d0 + 128])
        # [1536, D] -> transposed [D, 1536] -> [p, hp, a, 384]
        wqkvt = np.concatenate(rows, axis=0).T.astype(np.float16)
        wqkvt = np.ascontiguousarray(
            wqkvt.reshape(NDT, 128, NHP, 384).transpose(1, 2, 0, 3))
        # W_o columns for this head group, transposed [CD, D] -> [p, hp, D]
        wot = np.asarray(W_o[:, hg * 512:(hg + 1) * 512]).T.astype(np.float16)
        wot = np.ascontiguousarray(wot.reshape(NHP, 128, D).transpose(1, 0, 2))
        in_maps.append({"xt": xt, "wqkvt": wqkvt, "wot": wot, "mask": mask,
                        "onesd": onesd})

    if "nc" not in _NC_CACHE:
        _NC_CACHE["nc"] = _build_bass()
    nc = _NC_CACHE["nc"]

    trace = bool(os.environ.get("BASS_KERNEL_TRACE")) and _maybe_register_ntff_hook()
    res = run_bass_kernel_spmd(nc, in_maps, core_ids=list(range(N_CORES)),
                               trace=trace)
    if trace and res.exec_time_ns is not None:
        print(f"HW exec time: {res.exec_time_ns} ns")

    outs = []
    for b in range(B):
        outs.append(res.results[2 * b]["out"].astype(np.float32)
                    + res.results[2 * b + 1]["out"].astype(np.float32))
    return np.stack(outs).reshape(B, S, D)
